# revision 15
# baseline (speedup 1.0000x reference)
"""DynamicCenterLoss on Trainium2 (Bass/Tile), 8-core SPMD.

Strategy: `batch` is sorted, so shard at batch boundaries -> core b owns
batch b (sizes ~N/8 +- <1%). Per core, every needed statistic is a
13-class one-hot segment reduction computed on the tensor engine:

    OUT[13, 65] = sum_n onehot(tgt_n)^T (x) [feat_n | 1]
      -> fsum[13,64] (per-class feature sums), ccnt[13] (per-class counts)

plus S = sum_n ||feat_n||^2 via ScalarE Square+accumulate.  The intra
term uses  sum_n ||f_n - c_{t_n}||^2 = S - 2*sum_c c_c.fsum_c + sum_c
ccnt_c*||c_c||^2, so no per-point gather of centers is ever needed.
Padded rows (target=13) produce an all-zero one-hot row and zero
features, so they contribute nothing. The pairwise-center hinge loss is
computed per core on its own (13,64) stats; the host only averages the
8 per-batch scalars.
"""

import numpy as np

import concourse.bass as bass
import concourse.bacc as bacc
import concourse.tile as tile
from concourse import mybir
from concourse.bass_utils import run_bass_kernel_spmd

P = 128
D = 64
C = 13
B = 8
N_CORES = 8
MARGIN = 0.5
INTRA_W = 1.0
INTER_W = 1.0
LOSS_W = 0.01
IGNORE = -1
TT = 64  # matmul steps (128-point chunks) per SBUF tile

f32 = mybir.dt.float32
i32 = mybir.dt.int32


def build_nc(T: int) -> bass.Bass:
    """Build the per-core Bass program. T = points per SBUF partition."""
    Npad = P * T
    ntiles = (T + TT - 1) // TT

    nc = bacc.Bacc("TRN2", target_bir_lowering=False)
    feat_h = nc.dram_tensor("feat", [Npad, D], f32, kind="ExternalInput")
    tgt_h = nc.dram_tensor("tgt", [Npad], i32, kind="ExternalInput")
    cen_h = nc.dram_tensor("centers", [C, D], f32, kind="ExternalInput")
    out_h = nc.dram_tensor("out", [1, 8], f32, kind="ExternalOutput")

    # point n == (p, t) with n = p*T + t  -> per-partition contiguous DMA
    featv = feat_h[:, :].rearrange("(p t) d -> p t d", p=P)  # [128, T, 64]
    tgtv = tgt_h[:].rearrange("(p t) -> p t", p=P)  # [128, T]

    with tile.TileContext(nc) as tc:
        with (
            tc.tile_pool(name="consts", bufs=1) as cp,
            tc.tile_pool(name="io", bufs=3) as iop,
            tc.tile_pool(name="oh", bufs=3) as ohp,
            tc.tile_pool(name="sq", bufs=2) as sqp,
            tc.tile_pool(name="acc", bufs=1, space="PSUM") as psa,
            tc.tile_pool(name="ps2", bufs=1, space="PSUM") as ps2,
            tc.tile_pool(name="fin", bufs=1) as fp,
        ):
            # ---- constants ----
            iota_rep = cp.tile([P, TT, C], i32)
            nc.gpsimd.iota(
                iota_rep[:, :, :], pattern=[[0, TT], [1, C]], base=0,
                channel_multiplier=0,
            )
            tgt_sb = cp.tile([P, T], i32)
            nc.sync.dma_start(out=tgt_sb[:, :], in_=tgtv[:, :])
            cen_sb = cp.tile([C, D], f32)
            nc.sync.dma_start(out=cen_sb[:, :], in_=cen_h[:, :])
            ones = cp.tile([P, 1], f32)
            nc.vector.memset(ones[:, :], 1.0)
            ident = cp.tile([C, C], f32)
            nc.vector.memset(ident[:, :], 1.0)
            nc.gpsimd.affine_select(
                out=ident[:, :], in_=ident[:, :],
                compare_op=mybir.AluOpType.is_equal, fill=0.0,
                base=0, pattern=[[-1, C]], channel_multiplier=1,
            )
            noteye = cp.tile([1, C, C], f32)
            nc.vector.memset(noteye[:, :, :], 0.0)
            nc.gpsimd.affine_select(
                out=noteye[:, :, :], in_=noteye[:, :, :],
                compare_op=mybir.AluOpType.is_equal, fill=1.0,
                base=0, pattern=[[1, C], [-1, C]], channel_multiplier=0,
            )
            sq_acc = cp.tile([P, ntiles], f32)

            # constants done; fence so hot-loop ops carry no cross-engine
            # waits from setup (some ISA structs allow only 1 sync wait)
            tc.strict_bb_all_engine_barrier()

            # ---- main loop: accumulate OUT[13, 65] over all points ----
            acc = psa.tile([C, D + 1], f32)
            step = 0
            for i in range(ntiles):
                t0 = i * TT
                tt = min(TT, T - t0)
                ext = iop.tile([P, TT, D + 1], f32, tag="ext")
                nc.vector.memset(ext[:, :tt, D : D + 1], 1.0)
                nc.sync.dma_start(
                    out=ext[:, :tt, 0:D], in_=featv[:, t0 : t0 + tt, :]
                )
                oh = ohp.tile([P, TT, C], f32, tag="oh")
                nc.vector.tensor_tensor(
                    out=oh[:, :tt, :],
                    in0=tgt_sb[:, t0 : t0 + tt].unsqueeze(2).to_broadcast(
                        [P, tt, C]
                    ),
                    in1=iota_rep[:, :tt, :],
                    op=mybir.AluOpType.is_equal,
                )
                sq = sqp.tile([P, TT, D], f32, tag="sq")
                nc.scalar.activation(
                    out=sq[:, :tt, :], in_=ext[:, :tt, 0:D],
                    func=mybir.ActivationFunctionType.Square,
                    accum_out=sq_acc[:, i : i + 1],
                )
                for t in range(tt):
                    nc.tensor.matmul(
                        acc[:, :],
                        lhsT=oh[:, t, :],
                        rhs=ext[:, t, :],
                        start=(step == 0),
                        stop=(step == T - 1),
                    )
                    step += 1

            # ---- finale (tiny, per-core) ----
            fsum = acc[:, 0:D]  # [13, 64]
            ccnt = acc[:, D : D + 1]  # [13, 1]

            # per-class means and presence
            cmax = fp.tile([C, 1], f32)
            nc.vector.tensor_scalar(
                out=cmax[:, :], in0=ccnt, scalar1=1.0, scalar2=None,
                op0=mybir.AluOpType.max,
            )
            rec = fp.tile([C, 1], f32)
            nc.vector.reciprocal(rec[:, :], cmax[:, :])
            trin = fp.tile([C, D], f32)
            nc.vector.tensor_scalar(
                out=trin[:, :], in0=fsum, scalar1=rec[:, :], scalar2=None,
                op0=mybir.AluOpType.mult,
            )
            pres = fp.tile([C, 1], f32)
            nc.vector.tensor_scalar(
                out=pres[:, :], in0=ccnt, scalar1=0.0,
                scalar2=None, op0=mybir.AluOpType.is_gt,
            )

            # per-class dot(centers, fsum), ccnt*||c||^2  -> pack3
            junk0 = fp.tile([C, D], f32)
            cn2 = fp.tile([C, 1], f32)
            nc.vector.tensor_tensor(
                out=junk0[:, :], in0=cen_sb[:, :], in1=cen_sb[:, :],
                op=mybir.AluOpType.mult,
            )
            nc.vector.tensor_reduce(
                out=cn2[:, :], in_=junk0[:, :],
                axis=mybir.AxisListType.X, op=mybir.AluOpType.add,
            )
            pack3 = fp.tile([C, 3], f32)
            junk1 = fp.tile([C, D], f32)
            nc.vector.tensor_tensor(
                out=junk1[:, :], in0=cen_sb[:, :], in1=fsum,
                op=mybir.AluOpType.mult,
            )
            nc.vector.tensor_reduce(
                out=pack3[:, 0:1], in_=junk1[:, :],
                axis=mybir.AxisListType.X, op=mybir.AluOpType.add,
            )
            nc.vector.tensor_tensor(
                out=pack3[:, 1:2], in0=cn2[:, :], in1=ccnt,
                op=mybir.AluOpType.mult,
            )
            nc.vector.tensor_copy(pack3[:, 2:3], ccnt)

            # cross-partition sums over the 13 classes: [Tdot, Utot, cnt_b]
            red3 = ps2.tile([1, 3], f32)
            nc.tensor.matmul(
                red3[:, :], lhsT=ones[0:C, :], rhs=pack3[:, :],
                start=True, stop=True,
            )

            # S = sum over all partitions/tiles of sq_acc
            red_sq = fp.tile([P, 1], f32)
            nc.vector.tensor_reduce(
                out=red_sq[:, :], in_=sq_acc[:, :],
                axis=mybir.AxisListType.X, op=mybir.AluOpType.add,
            )
            s_ps = ps2.tile([1, 1], f32)
            nc.tensor.matmul(
                s_ps[:, :], lhsT=ones[:, :], rhs=red_sq[:, :],
                start=True, stop=True,
            )

            # transpose cmeans -> [64, 13]; present -> [1, 13]
            trps = ps2.tile([D, C], f32)
            nc.tensor.transpose(trps[:, :], trin[:, :], ident[:, :])
            trsb = fp.tile([D, C], f32)
            nc.vector.tensor_copy(trsb[:, :], trps[:, :])
            cmT = trsb[0:D, :]  # [64, 13]
            prps = ps2.tile([1, C], f32)
            nc.tensor.transpose(prps[:, :], pres[:, :], ident[:, :])
            presT = fp.tile([1, C], f32)
            nc.vector.tensor_copy(presT[:, :], prps[:, :])

            # pairwise squared distances between class means
            diff = fp.tile([D, C, C], f32)
            nc.vector.tensor_tensor(
                out=diff[:, :, :],
                in0=cmT.unsqueeze(2).to_broadcast([D, C, C]),
                in1=cmT.unsqueeze(1).to_broadcast([D, C, C]),
                op=mybir.AluOpType.subtract,
            )
            dsq = fp.tile([D, C, C], f32)
            nc.vector.tensor_tensor(
                out=dsq[:, :, :], in0=diff[:, :, :], in1=diff[:, :, :],
                op=mybir.AluOpType.mult,
            )
            dd2 = ps2.tile([1, C * C], f32)
            nc.tensor.matmul(
                dd2[:, :], lhsT=ones[0:D, :],
                rhs=dsq[:, :, :].rearrange("d a b -> d (a b)"),
                start=True, stop=True,
            )
            dist = fp.tile([1, C * C], f32)
            nc.scalar.activation(
                out=dist[:, :], in_=dd2[:, :],
                func=mybir.ActivationFunctionType.Sqrt,
            )
            marg = fp.tile([1, 1], f32)
            nc.vector.memset(marg[:, :], MARGIN)
            hinge = fp.tile([1, C * C], f32)
            nc.scalar.activation(
                out=hinge[:, :], in_=dist[:, :],
                func=mybir.ActivationFunctionType.Relu,
                bias=marg[:, :], scale=-1.0,
            )
            pm = fp.tile([1, C, C], f32)
            nc.vector.tensor_tensor(
                out=pm[:, :, :],
                in0=presT[:, :].unsqueeze(2).to_broadcast([1, C, C]),
                in1=presT[:, :].unsqueeze(1).to_broadcast([1, C, C]),
                op=mybir.AluOpType.mult,
            )
            nc.vector.tensor_tensor(
                out=pm[:, :, :], in0=pm[:, :, :], in1=noteye[:, :, :],
                op=mybir.AluOpType.mult,
            )
            pmf = pm[:, :, :].rearrange("p a b -> p (a b)")
            terms = fp.tile([1, C * C], f32)
            tsum = fp.tile([1, 1], f32)
            nc.vector.tensor_tensor(
                out=terms[:, :], in0=hinge[:, :], in1=pmf,
                op=mybir.AluOpType.mult,
            )
            nc.vector.tensor_reduce(
                out=tsum[:, :], in_=terms[:, :],
                axis=mybir.AxisListType.X, op=mybir.AluOpType.add,
            )
            npairs = fp.tile([1, 1], f32)
            nc.vector.tensor_reduce(
                out=npairs[:, :], in_=pmf, axis=mybir.AxisListType.X,
                op=mybir.AluOpType.add,
            )

            # scalar assembly on partition 0
            red3_sb = fp.tile([1, 3], f32)
            nc.vector.tensor_copy(red3_sb[:, :], red3[:, :])
            s_sb = fp.tile([1, 1], f32)
            nc.vector.tensor_copy(s_sb[:, :], s_ps[:, :])

            scal = fp.tile([1, 8], f32)
            t2 = fp.tile([1, 1], f32)
            nc.vector.tensor_scalar(
                out=t2[:, :], in0=red3_sb[:, 0:1], scalar1=-2.0,
                scalar2=None, op0=mybir.AluOpType.mult,
            )
            nc.vector.tensor_tensor(
                out=t2[:, :], in0=t2[:, :], in1=s_sb[:, :],
                op=mybir.AluOpType.add,
            )
            nc.vector.tensor_tensor(
                out=t2[:, :], in0=t2[:, :], in1=red3_sb[:, 1:2],
                op=mybir.AluOpType.add,
            )
            cbm = fp.tile([1, 1], f32)
            nc.vector.tensor_scalar(
                out=cbm[:, :], in0=red3_sb[:, 2:3], scalar1=1.0,
                scalar2=None, op0=mybir.AluOpType.max,
            )
            rcb = fp.tile([1, 1], f32)
            nc.vector.reciprocal(rcb[:, :], cbm[:, :])
            nc.vector.tensor_tensor(
                out=scal[:, 0:1], in0=t2[:, :], in1=rcb[:, :],
                op=mybir.AluOpType.mult,
            )
            npm = fp.tile([1, 1], f32)
            nc.vector.tensor_scalar(
                out=npm[:, :], in0=npairs[:, :], scalar1=1.0,
                scalar2=None, op0=mybir.AluOpType.max,
            )
            rnp = fp.tile([1, 1], f32)
            nc.vector.reciprocal(rnp[:, :], npm[:, :])
            nc.vector.tensor_tensor(
                out=scal[:, 1:2], in0=tsum[:, :], in1=rnp[:, :],
                op=mybir.AluOpType.mult,
            )
            nc.vector.tensor_scalar(
                out=scal[:, 2:3], in0=red3_sb[:, 2:3], scalar1=0.0,
                scalar2=None, op0=mybir.AluOpType.is_gt,
            )
            nc.vector.tensor_copy(scal[:, 3:4], red3_sb[:, 2:3])
            nc.vector.tensor_copy(scal[:, 4:5], s_sb[:, :])
            nc.vector.tensor_copy(scal[:, 5:6], red3_sb[:, 0:1])
            nc.vector.tensor_copy(scal[:, 6:7], red3_sb[:, 1:2])
            nc.vector.tensor_copy(scal[:, 7:8], npairs[:, :])

            nc.sync.dma_start(out=out_h[:, :], in_=scal[:, :])
    nc.finalize()
    return nc


# set by test.py to capture profile info
TRACE = False
LAST = {}


def _ensure_ntff_hook():
    """The agent image's antenv lacks axon_hooks; synthesize it so
    run_bass_kernel_spmd(trace=True) can profile. Best-effort."""
    import sys
    import types

    try:
        from antenv.axon_hooks import get_axon_ntff_profile_hook  # noqa: F401
        return
    except ImportError:
        pass
    try:
        from trn_agent_boot.trn_boot import _ntff_profile_via_ctypes

        hook = _ntff_profile_via_ctypes("/opt/axon/libaxon_pjrt.so")
        mod = types.ModuleType("antenv.axon_hooks")
        mod._hook = hook
        mod.get_axon_ntff_profile_hook = lambda: mod._hook
        mod.set_axon_ntff_profile_hook = lambda h: setattr(mod, "_hook", h)
        sys.modules["antenv.axon_hooks"] = mod
        import antenv

        antenv.axon_hooks = mod
    except Exception as e:  # degrade: no profile, run still works
        print(f"ntff hook injection failed: {e}")


def kernel(pred=None, target=None, feat=None, batch=None, centers=None):
    target = np.asarray(target)
    feat = np.asarray(feat, dtype=np.float32)
    batch = np.asarray(batch)
    centers = np.asarray(centers, dtype=np.float32)
    N = feat.shape[0]

    # shard at batch boundaries: core b <- batch b (batch is sorted)
    bounds = np.searchsorted(batch, np.arange(B + 1))
    sizes = np.diff(bounds)
    T = int(max((int(sizes.max()) + P - 1) // P, TT))
    Npad = P * T

    in_maps = []
    for b in range(B):
        lo, hi = int(bounds[b]), int(bounds[b + 1])
        fb = np.zeros((Npad, D), dtype=np.float32)
        tb = np.full((Npad,), C, dtype=np.int32)
        fb[: hi - lo] = feat[lo:hi]
        tb[: hi - lo] = target[lo:hi]
        inv = tb == IGNORE
        if inv.any():
            tb[inv] = C  # one-hot miss -> excluded everywhere
            fb[inv] = 0.0  # excluded from S
        in_maps.append({"feat": fb, "tgt": tb, "centers": centers})

    nc = build_nc(T)
    if TRACE:
        _ensure_ntff_hook()
    res = run_bass_kernel_spmd(nc, in_maps, list(range(N_CORES)), trace=TRACE)
    LAST["results"] = res

    rows = np.stack([np.asarray(res.results[b]["out"]).reshape(8) for b in range(B)])
    intra, inter, present = rows[:, 0], rows[:, 1], rows[:, 2]
    den = max(float(present.sum()), 1.0)
    loss = LOSS_W * (
        INTRA_W * float(np.where(present > 0, intra, 0.0).sum()) / den
        + INTER_W * float(np.where(present > 0, inter, 0.0).sum()) / den
    )
    return np.float32(loss)


# revision 17
# speedup vs baseline: 1.4893x; 1.4893x over previous
"""DynamicCenterLoss on Trainium2 (Bass/Tile), 8-core SPMD.

Strategy: `batch` is sorted, so shard at batch boundaries -> core b owns
batch b (sizes ~N/8 +- <1%). Per core, every needed statistic is a
13-class one-hot segment reduction computed on the tensor engine:

    OUT[13, 65] = sum_n onehot(tgt_n)^T (x) [feat_n | 1]
      -> fsum[13,64] (per-class feature sums), ccnt[13] (per-class counts)

plus S = sum_n ||feat_n||^2 via ScalarE Square+accumulate.  The intra
term uses  sum_n ||f_n - c_{t_n}||^2 = S - 2*sum_c c_c.fsum_c + sum_c
ccnt_c*||c_c||^2, so no per-point gather of centers is ever needed.
Padded rows (target=13) produce an all-zero one-hot row and zero
features, so they contribute nothing. The pairwise-center hinge loss is
computed per core on its own (13,64) stats; the host only averages the
8 per-batch scalars.
"""

import numpy as np

import concourse.bass as bass
import concourse.bacc as bacc
import concourse.tile as tile
from concourse import mybir
from concourse.bass_utils import run_bass_kernel_spmd

P = 128
D = 64
C = 13
B = 8
N_CORES = 8
MARGIN = 0.5
INTRA_W = 1.0
INTER_W = 1.0
LOSS_W = 0.01
IGNORE = -1
TT = 64  # matmul steps (128-point chunks) per SBUF tile

f32 = mybir.dt.float32
bf16 = mybir.dt.bfloat16
i32 = mybir.dt.int32


def build_nc(T: int) -> bass.Bass:
    """Build the per-core Bass program. T = points per SBUF partition."""
    Npad = P * T
    ntiles = (T + TT - 1) // TT

    nc = bacc.Bacc("TRN2", target_bir_lowering=False)
    feat_h = nc.dram_tensor("feat", [Npad, D], f32, kind="ExternalInput")
    tgt_h = nc.dram_tensor("tgt", [Npad], i32, kind="ExternalInput")
    cen_h = nc.dram_tensor("centers", [C, D], f32, kind="ExternalInput")
    out_h = nc.dram_tensor("out", [1, 8], f32, kind="ExternalOutput")

    # point n == (p, t) with n = p*T + t  -> per-partition contiguous DMA
    featv = feat_h[:, :].rearrange("(p t) d -> p t d", p=P)  # [128, T, 64]
    tgtv = tgt_h[:].rearrange("(p t) -> p t", p=P)  # [128, T]

    with tile.TileContext(nc) as tc:
        with (
            tc.tile_pool(name="consts", bufs=1) as cp,
            tc.tile_pool(name="io", bufs=3) as iop,
            tc.tile_pool(name="oh", bufs=3) as ohp,
            tc.tile_pool(name="sq", bufs=2) as sqp,
            tc.tile_pool(name="acc", bufs=1, space="PSUM") as psa,
            tc.tile_pool(name="ps2", bufs=1, space="PSUM") as ps2,
            tc.tile_pool(name="fin", bufs=1) as fp,
        ):
            # ---- constants ----
            iota_rep = cp.tile([P, TT, C], i32)
            nc.gpsimd.iota(
                iota_rep[:, :, :], pattern=[[0, TT], [1, C]], base=0,
                channel_multiplier=0,
            )
            tgt_sb = cp.tile([P, T], i32)
            nc.sync.dma_start(out=tgt_sb[:, :], in_=tgtv[:, :])
            cen_sb = cp.tile([C, D], f32)
            nc.sync.dma_start(out=cen_sb[:, :], in_=cen_h[:, :])
            ones = cp.tile([P, 1], f32)
            nc.vector.memset(ones[:, :], 1.0)
            ident = cp.tile([C, C], f32)
            nc.vector.memset(ident[:, :], 1.0)
            nc.gpsimd.affine_select(
                out=ident[:, :], in_=ident[:, :],
                compare_op=mybir.AluOpType.is_equal, fill=0.0,
                base=0, pattern=[[-1, C]], channel_multiplier=1,
            )
            noteye = cp.tile([1, C, C], f32)
            nc.vector.memset(noteye[:, :, :], 0.0)
            nc.gpsimd.affine_select(
                out=noteye[:, :, :], in_=noteye[:, :, :],
                compare_op=mybir.AluOpType.is_equal, fill=1.0,
                base=0, pattern=[[1, C], [-1, C]], channel_multiplier=0,
            )
            sq_acc = cp.tile([P, ntiles], f32)

            # constants done; fence so hot-loop ops carry no cross-engine
            # waits from setup (some ISA structs allow only 1 sync wait)
            tc.strict_bb_all_engine_barrier()

            # ---- main loop: accumulate OUT[13, 65] over all points ----
            acc = psa.tile([C, D + 1], f32)
            step = 0
            for i in range(ntiles):
                t0 = i * TT
                tt = min(TT, T - t0)
                # dense f32 load (16KB+ contiguous per partition), then
                # DVE-cast to bf16 into the [feat | 1] layout for the PE
                f32t = iop.tile([P, TT, D], f32, tag="f32t")
                nc.sync.dma_start(
                    out=f32t[:, :tt, :], in_=featv[:, t0 : t0 + tt, :]
                )
                ext = iop.tile([P, TT, D + 1], bf16, tag="ext")
                nc.vector.memset(ext[:, :tt, D : D + 1], 1.0)
                nc.vector.tensor_copy(ext[:, :tt, 0:D], f32t[:, :tt, :])
                oh = ohp.tile([P, TT, C], bf16, tag="oh")
                nc.vector.tensor_tensor(
                    out=oh[:, :tt, :],
                    in0=tgt_sb[:, t0 : t0 + tt].unsqueeze(2).to_broadcast(
                        [P, tt, C]
                    ),
                    in1=iota_rep[:, :tt, :],
                    op=mybir.AluOpType.is_equal,
                )
                sq = sqp.tile([P, TT, D], bf16, tag="sq")
                nc.scalar.activation(
                    out=sq[:, :tt, :], in_=ext[:, :tt, 0:D],
                    func=mybir.ActivationFunctionType.Square,
                    accum_out=sq_acc[:, i : i + 1],
                )
                for t in range(tt):
                    nc.tensor.matmul(
                        acc[:, :],
                        lhsT=oh[:, t, :],
                        rhs=ext[:, t, :],
                        start=(step == 0),
                        stop=(step == T - 1),
                    )
                    step += 1

            # ---- finale (tiny, per-core) ----
            fsum = acc[:, 0:D]  # [13, 64]
            ccnt = acc[:, D : D + 1]  # [13, 1]

            # per-class means and presence
            cmax = fp.tile([C, 1], f32)
            nc.vector.tensor_scalar(
                out=cmax[:, :], in0=ccnt, scalar1=1.0, scalar2=None,
                op0=mybir.AluOpType.max,
            )
            rec = fp.tile([C, 1], f32)
            nc.vector.reciprocal(rec[:, :], cmax[:, :])
            trin = fp.tile([C, D], f32)
            nc.vector.tensor_scalar(
                out=trin[:, :], in0=fsum, scalar1=rec[:, :], scalar2=None,
                op0=mybir.AluOpType.mult,
            )
            pres = fp.tile([C, 1], f32)
            nc.vector.tensor_scalar(
                out=pres[:, :], in0=ccnt, scalar1=0.0,
                scalar2=None, op0=mybir.AluOpType.is_gt,
            )

            # per-class dot(centers, fsum), ccnt*||c||^2  -> pack3
            junk0 = fp.tile([C, D], f32)
            cn2 = fp.tile([C, 1], f32)
            nc.vector.tensor_tensor(
                out=junk0[:, :], in0=cen_sb[:, :], in1=cen_sb[:, :],
                op=mybir.AluOpType.mult,
            )
            nc.vector.tensor_reduce(
                out=cn2[:, :], in_=junk0[:, :],
                axis=mybir.AxisListType.X, op=mybir.AluOpType.add,
            )
            pack3 = fp.tile([C, 3], f32)
            junk1 = fp.tile([C, D], f32)
            nc.vector.tensor_tensor(
                out=junk1[:, :], in0=cen_sb[:, :], in1=fsum,
                op=mybir.AluOpType.mult,
            )
            nc.vector.tensor_reduce(
                out=pack3[:, 0:1], in_=junk1[:, :],
                axis=mybir.AxisListType.X, op=mybir.AluOpType.add,
            )
            nc.vector.tensor_tensor(
                out=pack3[:, 1:2], in0=cn2[:, :], in1=ccnt,
                op=mybir.AluOpType.mult,
            )
            nc.vector.tensor_copy(pack3[:, 2:3], ccnt)

            # cross-partition sums over the 13 classes: [Tdot, Utot, cnt_b]
            red3 = ps2.tile([1, 3], f32)
            nc.tensor.matmul(
                red3[:, :], lhsT=ones[0:C, :], rhs=pack3[:, :],
                start=True, stop=True,
            )

            # S = sum over all partitions/tiles of sq_acc
            red_sq = fp.tile([P, 1], f32)
            nc.vector.tensor_reduce(
                out=red_sq[:, :], in_=sq_acc[:, :],
                axis=mybir.AxisListType.X, op=mybir.AluOpType.add,
            )
            s_ps = ps2.tile([1, 1], f32)
            nc.tensor.matmul(
                s_ps[:, :], lhsT=ones[:, :], rhs=red_sq[:, :],
                start=True, stop=True,
            )

            # transpose cmeans -> [64, 13]; present -> [1, 13]
            trps = ps2.tile([D, C], f32)
            nc.tensor.transpose(trps[:, :], trin[:, :], ident[:, :])
            trsb = fp.tile([D, C], f32)
            nc.vector.tensor_copy(trsb[:, :], trps[:, :])
            cmT = trsb[0:D, :]  # [64, 13]
            prps = ps2.tile([1, C], f32)
            nc.tensor.transpose(prps[:, :], pres[:, :], ident[:, :])
            presT = fp.tile([1, C], f32)
            nc.vector.tensor_copy(presT[:, :], prps[:, :])

            # pairwise squared distances between class means
            diff = fp.tile([D, C, C], f32)
            nc.vector.tensor_tensor(
                out=diff[:, :, :],
                in0=cmT.unsqueeze(2).to_broadcast([D, C, C]),
                in1=cmT.unsqueeze(1).to_broadcast([D, C, C]),
                op=mybir.AluOpType.subtract,
            )
            dsq = fp.tile([D, C, C], f32)
            nc.vector.tensor_tensor(
                out=dsq[:, :, :], in0=diff[:, :, :], in1=diff[:, :, :],
                op=mybir.AluOpType.mult,
            )
            dd2 = ps2.tile([1, C * C], f32)
            nc.tensor.matmul(
                dd2[:, :], lhsT=ones[0:D, :],
                rhs=dsq[:, :, :].rearrange("d a b -> d (a b)"),
                start=True, stop=True,
            )
            dist = fp.tile([1, C * C], f32)
            nc.scalar.activation(
                out=dist[:, :], in_=dd2[:, :],
                func=mybir.ActivationFunctionType.Sqrt,
            )
            marg = fp.tile([1, 1], f32)
            nc.vector.memset(marg[:, :], MARGIN)
            hinge = fp.tile([1, C * C], f32)
            nc.scalar.activation(
                out=hinge[:, :], in_=dist[:, :],
                func=mybir.ActivationFunctionType.Relu,
                bias=marg[:, :], scale=-1.0,
            )
            pm = fp.tile([1, C, C], f32)
            nc.vector.tensor_tensor(
                out=pm[:, :, :],
                in0=presT[:, :].unsqueeze(2).to_broadcast([1, C, C]),
                in1=presT[:, :].unsqueeze(1).to_broadcast([1, C, C]),
                op=mybir.AluOpType.mult,
            )
            nc.vector.tensor_tensor(
                out=pm[:, :, :], in0=pm[:, :, :], in1=noteye[:, :, :],
                op=mybir.AluOpType.mult,
            )
            pmf = pm[:, :, :].rearrange("p a b -> p (a b)")
            terms = fp.tile([1, C * C], f32)
            tsum = fp.tile([1, 1], f32)
            nc.vector.tensor_tensor(
                out=terms[:, :], in0=hinge[:, :], in1=pmf,
                op=mybir.AluOpType.mult,
            )
            nc.vector.tensor_reduce(
                out=tsum[:, :], in_=terms[:, :],
                axis=mybir.AxisListType.X, op=mybir.AluOpType.add,
            )
            npairs = fp.tile([1, 1], f32)
            nc.vector.tensor_reduce(
                out=npairs[:, :], in_=pmf, axis=mybir.AxisListType.X,
                op=mybir.AluOpType.add,
            )

            # scalar assembly on partition 0
            red3_sb = fp.tile([1, 3], f32)
            nc.vector.tensor_copy(red3_sb[:, :], red3[:, :])
            s_sb = fp.tile([1, 1], f32)
            nc.vector.tensor_copy(s_sb[:, :], s_ps[:, :])

            scal = fp.tile([1, 8], f32)
            t2 = fp.tile([1, 1], f32)
            nc.vector.tensor_scalar(
                out=t2[:, :], in0=red3_sb[:, 0:1], scalar1=-2.0,
                scalar2=None, op0=mybir.AluOpType.mult,
            )
            nc.vector.tensor_tensor(
                out=t2[:, :], in0=t2[:, :], in1=s_sb[:, :],
                op=mybir.AluOpType.add,
            )
            nc.vector.tensor_tensor(
                out=t2[:, :], in0=t2[:, :], in1=red3_sb[:, 1:2],
                op=mybir.AluOpType.add,
            )
            cbm = fp.tile([1, 1], f32)
            nc.vector.tensor_scalar(
                out=cbm[:, :], in0=red3_sb[:, 2:3], scalar1=1.0,
                scalar2=None, op0=mybir.AluOpType.max,
            )
            rcb = fp.tile([1, 1], f32)
            nc.vector.reciprocal(rcb[:, :], cbm[:, :])
            nc.vector.tensor_tensor(
                out=scal[:, 0:1], in0=t2[:, :], in1=rcb[:, :],
                op=mybir.AluOpType.mult,
            )
            npm = fp.tile([1, 1], f32)
            nc.vector.tensor_scalar(
                out=npm[:, :], in0=npairs[:, :], scalar1=1.0,
                scalar2=None, op0=mybir.AluOpType.max,
            )
            rnp = fp.tile([1, 1], f32)
            nc.vector.reciprocal(rnp[:, :], npm[:, :])
            nc.vector.tensor_tensor(
                out=scal[:, 1:2], in0=tsum[:, :], in1=rnp[:, :],
                op=mybir.AluOpType.mult,
            )
            nc.vector.tensor_scalar(
                out=scal[:, 2:3], in0=red3_sb[:, 2:3], scalar1=0.0,
                scalar2=None, op0=mybir.AluOpType.is_gt,
            )
            nc.vector.tensor_copy(scal[:, 3:4], red3_sb[:, 2:3])
            nc.vector.tensor_copy(scal[:, 4:5], s_sb[:, :])
            nc.vector.tensor_copy(scal[:, 5:6], red3_sb[:, 0:1])
            nc.vector.tensor_copy(scal[:, 6:7], red3_sb[:, 1:2])
            nc.vector.tensor_copy(scal[:, 7:8], npairs[:, :])

            nc.sync.dma_start(out=out_h[:, :], in_=scal[:, :])
    nc.finalize()
    return nc


# set by test.py to capture profile info
TRACE = False
LAST = {}


def _ensure_ntff_hook():
    """The agent image's antenv lacks axon_hooks; synthesize it so
    run_bass_kernel_spmd(trace=True) can profile. Best-effort."""
    import sys
    import types

    try:
        from antenv.axon_hooks import get_axon_ntff_profile_hook  # noqa: F401
        return
    except ImportError:
        pass
    try:
        from trn_agent_boot.trn_boot import _ntff_profile_via_ctypes

        hook = _ntff_profile_via_ctypes("/opt/axon/libaxon_pjrt.so")
        mod = types.ModuleType("antenv.axon_hooks")
        mod._hook = hook
        mod.get_axon_ntff_profile_hook = lambda: mod._hook
        mod.set_axon_ntff_profile_hook = lambda h: setattr(mod, "_hook", h)
        sys.modules["antenv.axon_hooks"] = mod
        import antenv

        antenv.axon_hooks = mod
    except Exception as e:  # degrade: no profile, run still works
        print(f"ntff hook injection failed: {e}")


def kernel(pred=None, target=None, feat=None, batch=None, centers=None):
    target = np.asarray(target)
    feat = np.asarray(feat, dtype=np.float32)
    batch = np.asarray(batch)
    centers = np.asarray(centers, dtype=np.float32)
    N = feat.shape[0]

    # shard at batch boundaries: core b <- batch b (batch is sorted)
    bounds = np.searchsorted(batch, np.arange(B + 1))
    sizes = np.diff(bounds)
    T = int(max((int(sizes.max()) + P - 1) // P, TT))
    Npad = P * T

    in_maps = []
    for b in range(B):
        lo, hi = int(bounds[b]), int(bounds[b + 1])
        fb = np.zeros((Npad, D), dtype=np.float32)
        tb = np.full((Npad,), C, dtype=np.int32)
        fb[: hi - lo] = feat[lo:hi]
        tb[: hi - lo] = target[lo:hi]
        inv = tb == IGNORE
        if inv.any():
            tb[inv] = C  # one-hot miss -> excluded everywhere
            fb[inv] = 0.0  # excluded from S
        in_maps.append({"feat": fb, "tgt": tb, "centers": centers})

    nc = build_nc(T)
    if TRACE:
        _ensure_ntff_hook()
    res = run_bass_kernel_spmd(nc, in_maps, list(range(N_CORES)), trace=TRACE)
    LAST["results"] = res

    rows = np.stack([np.asarray(res.results[b]["out"]).reshape(8) for b in range(B)])
    intra, inter, present = rows[:, 0], rows[:, 1], rows[:, 2]
    den = max(float(present.sum()), 1.0)
    loss = LOSS_W * (
        INTRA_W * float(np.where(present > 0, intra, 0.0).sum()) / den
        + INTER_W * float(np.where(present > 0, inter, 0.0).sum()) / den
    )
    return np.float32(loss)


# revision 20
# speedup vs baseline: 1.5718x; 1.0554x over previous
"""DynamicCenterLoss on Trainium2 (Bass/Tile), 8-core SPMD.

Strategy: `batch` is sorted, so shard at batch boundaries -> core b owns
batch b (sizes ~N/8 +- <1%). Per core, every needed statistic is a
13-class one-hot segment reduction computed on the tensor engine:

    OUT[13, 65] = sum_n onehot(tgt_n)^T (x) [feat_n | 1]
      -> fsum[13,64] (per-class feature sums), ccnt[13] (per-class counts)

plus S = sum_n ||feat_n||^2 via ScalarE Square+accumulate.  The intra
term uses  sum_n ||f_n - c_{t_n}||^2 = S - 2*sum_c c_c.fsum_c + sum_c
ccnt_c*||c_c||^2, so no per-point gather of centers is ever needed.
Padded rows (target=13) produce an all-zero one-hot row and zero
features, so they contribute nothing. The pairwise-center hinge loss is
computed per core on its own (13,64) stats; the host only averages the
8 per-batch scalars.
"""

import numpy as np

import concourse.bass as bass
import concourse.bacc as bacc
import concourse.tile as tile
from concourse import mybir
from concourse.bass_utils import run_bass_kernel_spmd

P = 128
D = 64
C = 13
B = 8
N_CORES = 8
MARGIN = 0.5
INTRA_W = 1.0
INTER_W = 1.0
LOSS_W = 0.01
IGNORE = -1
TT = 64  # matmul steps (128-point chunks) per SBUF tile

f32 = mybir.dt.float32
bf16 = mybir.dt.bfloat16
i32 = mybir.dt.int32


def build_nc(T: int) -> bass.Bass:
    """Build the per-core Bass program. T = points per SBUF partition."""
    Npad = P * T
    ntiles = (T + TT - 1) // TT

    nc = bacc.Bacc("TRN2", target_bir_lowering=False)
    feat_h = nc.dram_tensor("feat", [Npad, D], f32, kind="ExternalInput")
    tgt_h = nc.dram_tensor("tgt", [Npad], i32, kind="ExternalInput")
    cen_h = nc.dram_tensor("centers", [C, D], f32, kind="ExternalInput")
    out_h = nc.dram_tensor("out", [1, 8], f32, kind="ExternalOutput")

    # point n == (p, t) with n = p*T + t  -> per-partition contiguous DMA
    featv = feat_h[:, :].rearrange("(p t) d -> p t d", p=P)  # [128, T, 64]
    tgtv = tgt_h[:].rearrange("(p t) -> p t", p=P)  # [128, T]

    with tile.TileContext(nc) as tc:
        with (
            tc.tile_pool(name="consts", bufs=1) as cp,
            tc.tile_pool(name="io", bufs=4) as iop,
            tc.tile_pool(name="oh", bufs=3) as ohp,
            tc.tile_pool(name="sq", bufs=2) as sqp,
            tc.tile_pool(name="acc", bufs=1, space="PSUM") as psa,
            tc.tile_pool(name="ps2", bufs=1, space="PSUM") as ps2,
            tc.tile_pool(name="fin", bufs=1) as fp,
        ):
            # ---- constants ----
            iota_rep = cp.tile([P, TT, C], i32)
            nc.gpsimd.iota(
                iota_rep[:, :, :], pattern=[[0, TT], [1, C]], base=0,
                channel_multiplier=0,
            )
            tgt_sb = cp.tile([P, T], i32)
            nc.sync.dma_start(out=tgt_sb[:, :], in_=tgtv[:, :])
            cen_sb = cp.tile([C, D], f32)
            nc.sync.dma_start(out=cen_sb[:, :], in_=cen_h[:, :])
            ones = cp.tile([P, 1], f32)
            nc.vector.memset(ones[:, :], 1.0)
            ident = cp.tile([C, C], f32)
            nc.vector.memset(ident[:, :], 1.0)
            nc.gpsimd.affine_select(
                out=ident[:, :], in_=ident[:, :],
                compare_op=mybir.AluOpType.is_equal, fill=0.0,
                base=0, pattern=[[-1, C]], channel_multiplier=1,
            )
            noteye = cp.tile([1, C, C], f32)
            nc.vector.memset(noteye[:, :, :], 0.0)
            nc.gpsimd.affine_select(
                out=noteye[:, :, :], in_=noteye[:, :, :],
                compare_op=mybir.AluOpType.is_equal, fill=1.0,
                base=0, pattern=[[1, C], [-1, C]], channel_multiplier=0,
            )
            sq_acc = cp.tile([P, ntiles], f32)

            # ---- main loop: accumulate OUT[13, 65] over all points ----
            acc = psa.tile([C, D + 1], f32)
            step = 0
            for i in range(ntiles):
                t0 = i * TT
                tt = min(TT, T - t0)
                # dense f32 load (16KB+ contiguous per partition), then
                # DVE-cast to bf16 into the [feat | 1] layout for the PE
                f32t = iop.tile([P, TT, D], f32, tag="f32t")
                dma_eng = nc.sync if i % 2 == 0 else nc.scalar
                dma_eng.dma_start(
                    out=f32t[:, :tt, :], in_=featv[:, t0 : t0 + tt, :]
                )
                ext = iop.tile([P, TT, D + 1], bf16, tag="ext")
                nc.vector.memset(ext[:, :tt, D : D + 1], 1.0)
                nc.vector.tensor_copy(ext[:, :tt, 0:D], f32t[:, :tt, :])
                oh = ohp.tile([P, TT, C], bf16, tag="oh")
                nc.vector.tensor_tensor(
                    out=oh[:, :tt, :],
                    in0=tgt_sb[:, t0 : t0 + tt].unsqueeze(2).to_broadcast(
                        [P, tt, C]
                    ),
                    in1=iota_rep[:, :tt, :],
                    op=mybir.AluOpType.is_equal,
                )
                sq = sqp.tile([P, TT, D], bf16, tag="sq")
                nc.scalar.activation(
                    out=sq[:, :tt, :], in_=ext[:, :tt, 0:D],
                    func=mybir.ActivationFunctionType.Square,
                    accum_out=sq_acc[:, i : i + 1],
                )
                for t in range(tt):
                    nc.tensor.matmul(
                        acc[:, :],
                        lhsT=oh[:, t, :],
                        rhs=ext[:, t, :],
                        start=(step == 0),
                        stop=(step == T - 1),
                    )
                    step += 1

            # ---- finale (tiny, per-core) ----
            fsum = acc[:, 0:D]  # [13, 64]
            ccnt = acc[:, D : D + 1]  # [13, 1]

            # per-class means and presence
            cmax = fp.tile([C, 1], f32)
            nc.vector.tensor_scalar(
                out=cmax[:, :], in0=ccnt, scalar1=1.0, scalar2=None,
                op0=mybir.AluOpType.max,
            )
            rec = fp.tile([C, 1], f32)
            nc.vector.reciprocal(rec[:, :], cmax[:, :])
            trin = fp.tile([C, D], f32)
            nc.vector.tensor_scalar(
                out=trin[:, :], in0=fsum, scalar1=rec[:, :], scalar2=None,
                op0=mybir.AluOpType.mult,
            )
            pres = fp.tile([C, 1], f32)
            nc.vector.tensor_scalar(
                out=pres[:, :], in0=ccnt, scalar1=0.0,
                scalar2=None, op0=mybir.AluOpType.is_gt,
            )

            # per-class dot(centers, fsum), ccnt*||c||^2  -> pack3
            junk0 = fp.tile([C, D], f32)
            cn2 = fp.tile([C, 1], f32)
            nc.vector.tensor_tensor(
                out=junk0[:, :], in0=cen_sb[:, :], in1=cen_sb[:, :],
                op=mybir.AluOpType.mult,
            )
            nc.vector.tensor_reduce(
                out=cn2[:, :], in_=junk0[:, :],
                axis=mybir.AxisListType.X, op=mybir.AluOpType.add,
            )
            pack3 = fp.tile([C, 3], f32)
            junk1 = fp.tile([C, D], f32)
            nc.vector.tensor_tensor(
                out=junk1[:, :], in0=cen_sb[:, :], in1=fsum,
                op=mybir.AluOpType.mult,
            )
            nc.vector.tensor_reduce(
                out=pack3[:, 0:1], in_=junk1[:, :],
                axis=mybir.AxisListType.X, op=mybir.AluOpType.add,
            )
            nc.vector.tensor_tensor(
                out=pack3[:, 1:2], in0=cn2[:, :], in1=ccnt,
                op=mybir.AluOpType.mult,
            )
            nc.vector.tensor_copy(pack3[:, 2:3], ccnt)

            # cross-partition sums over the 13 classes: [Tdot, Utot, cnt_b]
            red3 = ps2.tile([1, 3], f32)
            nc.tensor.matmul(
                red3[:, :], lhsT=ones[0:C, :], rhs=pack3[:, :],
                start=True, stop=True,
            )

            # S = sum over all partitions/tiles of sq_acc
            red_sq = fp.tile([P, 1], f32)
            nc.vector.tensor_reduce(
                out=red_sq[:, :], in_=sq_acc[:, :],
                axis=mybir.AxisListType.X, op=mybir.AluOpType.add,
            )
            s_ps = ps2.tile([1, 1], f32)
            nc.tensor.matmul(
                s_ps[:, :], lhsT=ones[:, :], rhs=red_sq[:, :],
                start=True, stop=True,
            )

            # transpose cmeans -> [64, 13]; present -> [1, 13]
            trps = ps2.tile([D, C], f32)
            nc.tensor.transpose(trps[:, :], trin[:, :], ident[:, :])
            trsb = fp.tile([D, C], f32)
            nc.vector.tensor_copy(trsb[:, :], trps[:, :])
            cmT = trsb[0:D, :]  # [64, 13]
            prps = ps2.tile([1, C], f32)
            nc.tensor.transpose(prps[:, :], pres[:, :], ident[:, :])
            presT = fp.tile([1, C], f32)
            nc.vector.tensor_copy(presT[:, :], prps[:, :])

            # pairwise squared distances between class means
            diff = fp.tile([D, C, C], f32)
            nc.vector.tensor_tensor(
                out=diff[:, :, :],
                in0=cmT.unsqueeze(2).to_broadcast([D, C, C]),
                in1=cmT.unsqueeze(1).to_broadcast([D, C, C]),
                op=mybir.AluOpType.subtract,
            )
            dsq = fp.tile([D, C, C], f32)
            nc.vector.tensor_tensor(
                out=dsq[:, :, :], in0=diff[:, :, :], in1=diff[:, :, :],
                op=mybir.AluOpType.mult,
            )
            dd2 = ps2.tile([1, C * C], f32)
            nc.tensor.matmul(
                dd2[:, :], lhsT=ones[0:D, :],
                rhs=dsq[:, :, :].rearrange("d a b -> d (a b)"),
                start=True, stop=True,
            )
            dist = fp.tile([1, C * C], f32)
            nc.scalar.activation(
                out=dist[:, :], in_=dd2[:, :],
                func=mybir.ActivationFunctionType.Sqrt,
            )
            marg = fp.tile([1, 1], f32)
            nc.vector.memset(marg[:, :], MARGIN)
            hinge = fp.tile([1, C * C], f32)
            nc.scalar.activation(
                out=hinge[:, :], in_=dist[:, :],
                func=mybir.ActivationFunctionType.Relu,
                bias=marg[:, :], scale=-1.0,
            )
            pm = fp.tile([1, C, C], f32)
            nc.vector.tensor_tensor(
                out=pm[:, :, :],
                in0=presT[:, :].unsqueeze(2).to_broadcast([1, C, C]),
                in1=presT[:, :].unsqueeze(1).to_broadcast([1, C, C]),
                op=mybir.AluOpType.mult,
            )
            nc.vector.tensor_tensor(
                out=pm[:, :, :], in0=pm[:, :, :], in1=noteye[:, :, :],
                op=mybir.AluOpType.mult,
            )
            pmf = pm[:, :, :].rearrange("p a b -> p (a b)")
            terms = fp.tile([1, C * C], f32)
            tsum = fp.tile([1, 1], f32)
            nc.vector.tensor_tensor(
                out=terms[:, :], in0=hinge[:, :], in1=pmf,
                op=mybir.AluOpType.mult,
            )
            nc.vector.tensor_reduce(
                out=tsum[:, :], in_=terms[:, :],
                axis=mybir.AxisListType.X, op=mybir.AluOpType.add,
            )
            npairs = fp.tile([1, 1], f32)
            nc.vector.tensor_reduce(
                out=npairs[:, :], in_=pmf, axis=mybir.AxisListType.X,
                op=mybir.AluOpType.add,
            )

            # scalar assembly on partition 0
            red3_sb = fp.tile([1, 3], f32)
            nc.vector.tensor_copy(red3_sb[:, :], red3[:, :])
            s_sb = fp.tile([1, 1], f32)
            nc.vector.tensor_copy(s_sb[:, :], s_ps[:, :])

            scal = fp.tile([1, 8], f32)
            t2 = fp.tile([1, 1], f32)
            nc.vector.tensor_scalar(
                out=t2[:, :], in0=red3_sb[:, 0:1], scalar1=-2.0,
                scalar2=None, op0=mybir.AluOpType.mult,
            )
            nc.vector.tensor_tensor(
                out=t2[:, :], in0=t2[:, :], in1=s_sb[:, :],
                op=mybir.AluOpType.add,
            )
            nc.vector.tensor_tensor(
                out=t2[:, :], in0=t2[:, :], in1=red3_sb[:, 1:2],
                op=mybir.AluOpType.add,
            )
            cbm = fp.tile([1, 1], f32)
            nc.vector.tensor_scalar(
                out=cbm[:, :], in0=red3_sb[:, 2:3], scalar1=1.0,
                scalar2=None, op0=mybir.AluOpType.max,
            )
            rcb = fp.tile([1, 1], f32)
            nc.vector.reciprocal(rcb[:, :], cbm[:, :])
            nc.vector.tensor_tensor(
                out=scal[:, 0:1], in0=t2[:, :], in1=rcb[:, :],
                op=mybir.AluOpType.mult,
            )
            npm = fp.tile([1, 1], f32)
            nc.vector.tensor_scalar(
                out=npm[:, :], in0=npairs[:, :], scalar1=1.0,
                scalar2=None, op0=mybir.AluOpType.max,
            )
            rnp = fp.tile([1, 1], f32)
            nc.vector.reciprocal(rnp[:, :], npm[:, :])
            nc.vector.tensor_tensor(
                out=scal[:, 1:2], in0=tsum[:, :], in1=rnp[:, :],
                op=mybir.AluOpType.mult,
            )
            nc.vector.tensor_scalar(
                out=scal[:, 2:3], in0=red3_sb[:, 2:3], scalar1=0.0,
                scalar2=None, op0=mybir.AluOpType.is_gt,
            )
            nc.vector.tensor_copy(scal[:, 3:4], red3_sb[:, 2:3])
            nc.vector.tensor_copy(scal[:, 4:5], s_sb[:, :])
            nc.vector.tensor_copy(scal[:, 5:6], red3_sb[:, 0:1])
            nc.vector.tensor_copy(scal[:, 6:7], red3_sb[:, 1:2])
            nc.vector.tensor_copy(scal[:, 7:8], npairs[:, :])

            nc.sync.dma_start(out=out_h[:, :], in_=scal[:, :])
    nc.finalize()
    return nc


# set by test.py to capture profile info
TRACE = False
LAST = {}


def _ensure_ntff_hook():
    """The agent image's antenv lacks axon_hooks; synthesize it so
    run_bass_kernel_spmd(trace=True) can profile. Best-effort."""
    import sys
    import types

    try:
        from antenv.axon_hooks import get_axon_ntff_profile_hook  # noqa: F401
        return
    except ImportError:
        pass
    try:
        from trn_agent_boot.trn_boot import _ntff_profile_via_ctypes

        hook = _ntff_profile_via_ctypes("/opt/axon/libaxon_pjrt.so")
        mod = types.ModuleType("antenv.axon_hooks")
        mod._hook = hook
        mod.get_axon_ntff_profile_hook = lambda: mod._hook
        mod.set_axon_ntff_profile_hook = lambda h: setattr(mod, "_hook", h)
        sys.modules["antenv.axon_hooks"] = mod
        import antenv

        antenv.axon_hooks = mod
    except Exception as e:  # degrade: no profile, run still works
        print(f"ntff hook injection failed: {e}")


def kernel(pred=None, target=None, feat=None, batch=None, centers=None):
    target = np.asarray(target)
    feat = np.asarray(feat, dtype=np.float32)
    batch = np.asarray(batch)
    centers = np.asarray(centers, dtype=np.float32)
    N = feat.shape[0]

    # shard at batch boundaries: core b <- batch b (batch is sorted)
    bounds = np.searchsorted(batch, np.arange(B + 1))
    sizes = np.diff(bounds)
    T = int(max((int(sizes.max()) + P - 1) // P, TT))
    Npad = P * T

    in_maps = []
    for b in range(B):
        lo, hi = int(bounds[b]), int(bounds[b + 1])
        fb = np.zeros((Npad, D), dtype=np.float32)
        tb = np.full((Npad,), C, dtype=np.int32)
        fb[: hi - lo] = feat[lo:hi]
        tb[: hi - lo] = target[lo:hi]
        inv = tb == IGNORE
        if inv.any():
            tb[inv] = C  # one-hot miss -> excluded everywhere
            fb[inv] = 0.0  # excluded from S
        in_maps.append({"feat": fb, "tgt": tb, "centers": centers})

    nc = build_nc(T)
    if TRACE:
        _ensure_ntff_hook()
    res = run_bass_kernel_spmd(nc, in_maps, list(range(N_CORES)), trace=TRACE)
    LAST["results"] = res

    rows = np.stack([np.asarray(res.results[b]["out"]).reshape(8) for b in range(B)])
    intra, inter, present = rows[:, 0], rows[:, 1], rows[:, 2]
    den = max(float(present.sum()), 1.0)
    loss = LOSS_W * (
        INTRA_W * float(np.where(present > 0, intra, 0.0).sum()) / den
        + INTER_W * float(np.where(present > 0, inter, 0.0).sum()) / den
    )
    return np.float32(loss)
